# revision 22
# baseline (speedup 1.0000x reference)
"""MatAnyone memory-readout kernel for 8 Trainium2 NeuronCores.

Math (per batch b):
  sim[t,n]  = (-a_sq + two_ab - b_sq)[t,n] * ms[t] / sqrt(CK)
  aff       = softmax_t(sim)
  R[c,n]    = sum_t mv[c,t] * aff[t,n]
  out[c,n]  = R[c,n] * p[n] + lv[c,n] * (1 - p[n])

Sharding: 8 cores = 2 batches x 4 query-pixel shards (n = HW/4 = 576 each).
Single interleaved pass over 144 t-tiles. Per tile the PE issues 8 N=288
matmuls (two per weight load, all K=128 so the PE never changes tiling mode):
2 psum inits seeding -b_sq/8 (all-ones lhsT against a zero-padded rhs whose
row 0 holds -b_sq/8), 2 sim accumulates with lhsT = [mk^2 ; mk] against
rhs [-qe/8 ; qe*qk/4], and 4 readout accumulates. The two sim halves live at
[0:288] and [512:800] of one two-bank psum tile so a single strided ACT
computes E = Exp(ms_t * psum) for the whole tile (per-partition scale, PSUM
source). The softmax denominator Z is accumulated on the DVE (pairwise bf16
adds at 2x + f32 accumulate). A manual load of ACT table set 6
(natural_log_exp_and_others) covers Exp AND Ln so the finalize's
1/Z = exp(-ln Z) causes no table switches. Finalize: Z column-sum matmul,
out = R * (p/Z) + host-folded lv*(1-p). Softmax max-subtraction is skipped:
sim <= 0 always (negative weighted L2 distance), so exp never overflows and
Z is well-scaled.
"""

import sys

for _p in ("/opt/trn_rl_repo", "/root/.axon_site/_ro/trn_rl_repo"):
    if _p not in sys.path:
        sys.path.insert(0, _p)

from contextlib import ExitStack

import numpy as np
import ml_dtypes

import concourse.bass as bass
from concourse import mybir
from concourse.bacc import Bacc
from concourse.tile import TileContext
from concourse.bass_utils import run_bass_kernel_spmd

F32 = mybir.dt.float32
F32R = mybir.dt.float32r
BF16 = mybir.dt.bfloat16
FP16 = mybir.dt.float16
EXP = mybir.ActivationFunctionType.Exp
LOG = mybir.ActivationFunctionType.Ln

B, CK, CV, T, H, W = 2, 64, 256, 8, 48, 48
HW = H * W            # 2304
THW = T * HW          # 18432
NCORE = HW // 4       # 576 query pixels per core
NH = NCORE // 2       # 288 per n-half (psum-bank sized)
TT = THW // 128       # 144 t-tiles
SKEW = 4              # software-pipeline skew (tiles) between exp and readout
ACT_SET_LN_EXP = 6    # natural_log_exp_and_others in act_info.json

# mkw DMA pieces (start_tile, n_tiles): small first piece so sims start early
MK_PIECES = [(0, 8), (8, 48), (56, 44), (100, 44)]

_CACHE = {}


def build_program():
    nc = Bacc(name="matanyone_knn")

    cz_h = nc.declare_dram_parameter("c_onesz", [128, 2], F32R, isOutput=False)
    cb_h = nc.declare_dram_parameter("c_onesb", [1, 128], F32R, isOutput=False)
    qs_h = nc.declare_dram_parameter("qs", [128, NCORE], FP16, isOutput=False)
    nb_h = nc.declare_dram_parameter("nbsq", [1, NCORE], FP16, isOutput=False)
    mkw_h = nc.declare_dram_parameter("mkw", [128, THW], FP16, isOutput=False)
    ms_h = nc.declare_dram_parameter("msT", [128, TT], F32, isOutput=False)
    mv_h = nc.declare_dram_parameter("mvT", [THW, CV], FP16, isOutput=False)
    lvw_h = nc.declare_dram_parameter("lvw", [CV, NCORE], F32, isOutput=False)
    p_h = nc.declare_dram_parameter("p", [1, NCORE], F32, isOutput=False)
    out_h = nc.declare_dram_parameter("out", [CV, NCORE], F32, isOutput=True)

    with TileContext(nc) as tc, ExitStack() as ctx:
        persist = ctx.enter_context(tc.tile_pool(name="persist", bufs=1))
        mvpool = ctx.enter_context(tc.tile_pool(name="mv", bufs=1))
        mkpool = ctx.enter_context(tc.tile_pool(name="mkw", bufs=1))
        epool = ctx.enter_context(tc.tile_pool(name="E", bufs=SKEW + 4))
        spool = ctx.enter_context(tc.tile_pool(name="S", bufs=2))
        dpool = ctx.enter_context(tc.tile_pool(name="D", bufs=2))
        fin = ctx.enter_context(tc.tile_pool(name="fin", bufs=1))
        ps_sim = ctx.enter_context(tc.tile_pool(name="pssim", bufs=2, space="PSUM"))
        ps_acc = ctx.enter_context(tc.tile_pool(name="psacc", bufs=1, space="PSUM"))

        # preload the ACT table set that has BOTH Exp and Ln, so the
        # finalize's Ln/Exp trigger no mid-kernel table switches
        nc.scalar.add_instruction(mybir.InstLoadActFuncSet(
            name=nc.get_next_instruction_name(), ins=[], outs=[],
            act_func_set_id=ACT_SET_LN_EXP))

        # ---- streamed big weights on the gpsimd DMA queue ----------------
        mkw_sb = [mkpool.tile([128, n * 128], FP16, tag=f"mkw{i}",
                              name=f"mkw{i}")
                  for i, (s0, n) in enumerate(MK_PIECES)]

        def load_mkw_piece(i, eng=None):
            s0, n = MK_PIECES[i]
            (eng or nc.gpsimd).dma_start(
                out=mkw_sb[i][:], in_=mkw_h[:, s0 * 128:(s0 + n) * 128])

        mv_sb = mvpool.tile([128, TT * CV], FP16, tag="mvres")

        def load_mv_chunk(g):
            src = mv_h[g * 2048:(g + 1) * 2048, :].rearrange(
                "(j p) c -> p j c", p=128)
            dst = mv_sb[:, g * 16 * CV:(g + 1) * 16 * CV].rearrange(
                "p (j c) -> p j c", c=CV)
            nc.gpsimd.dma_start(out=dst, in_=src)

        # ---- constants generated on-device; small criticals first --------
        ones128 = persist.tile([128, 128], FP16, tag="ones128")  # init lhsT
        nc.vector.memset(ones128[:], 1.0)
        nbsqz = persist.tile([128, NCORE], FP16, tag="nbsqz")
        nc.vector.memset(nbsqz[:], 0.0)
        nc.gpsimd.dma_start(out=nbsqz[0:1, :], in_=nb_h[:])      # row 0 only

        # mkw piece0 + qs + ms ride the pre-loop-idle scalar DMA queue
        # (fast, big packets); piece1 follows them
        load_mkw_piece(0, nc.scalar)
        qs_sb = persist.tile([128, NCORE], FP16, tag="qs")
        nc.scalar.dma_start(out=qs_sb[:], in_=qs_h[:])
        ms_sb = persist.tile([128, TT], F32, tag="ms")
        nc.scalar.dma_start(out=ms_sb[:], in_=ms_h[:])
        load_mkw_piece(1, nc.scalar)

        load_mv_chunk(0)
        load_mv_chunk(1)

        ones_z = persist.tile([128, 2], F32R, tag="ones_z")      # Z matmul lhsT
        nc.sync.dma_start(out=ones_z[:], in_=cz_h[:])
        ones_b = persist.tile([1, 128], F32R, tag="ones_b")      # bcast lhsT
        nc.sync.dma_start(out=ones_b[:], in_=cb_h[:])
        p_sb = persist.tile([1, NCORE], F32, tag="p")
        nc.sync.dma_start(out=p_sb[:], in_=p_h[:])

        lv0 = fin.tile([128, NCORE], F32, tag="lv0")
        lv1 = fin.tile([128, NCORE], F32, tag="lv1")

        # -b_sq/8 broadcast to all partitions, for the DVE-subtract tiles
        bsqb = persist.tile([128, NCORE], F32, tag="bsqb")
        for hh in (0, 1):
            bp = ps_sim.tile([128, 1024], F32, tag="sim", name=f"bp{hh}")
            nc.tensor.matmul(bp[:, 0:NH], ones128[:],
                             nbsqz[:, hh * NH:(hh + 1) * NH],
                             start=True, stop=True)
            nc.vector.tensor_copy(bsqb[:, hh * NH:(hh + 1) * NH],
                                  bp[:, 0:NH])

        # ---- main interleaved pass --------------------------------------
        r_acc = {}
        for k in (0, 1):
            for hh in (0, 1):
                r_acc[k, hh] = ps_acc.tile([128, NH], F32, tag=f"r{k}{hh}",
                                           name=f"r{k}{hh}")

        zacc = persist.tile([128, NCORE], F32R, tag="zacc")

        # tile index -> (mkw piece, offset within piece)
        t2piece = {}
        for i, (s0, n) in enumerate(MK_PIECES):
            for j in range(n):
                t2piece[s0 + j] = (i, j)

        HOFF = (0, 512)   # column offsets of the two n-halves in sim psum

        e_tiles = {}
        for t in range(TT + SKEW):
            if t < TT:
                if t % 16 == 0 and 2 + t // 16 <= 8:
                    load_mv_chunk(2 + t // 16)
                if t == 20:
                    load_mkw_piece(2)
                elif t == 64:
                    load_mkw_piece(3)
                elif t == 30:
                    nc.gpsimd.dma_start(out=lv0[:], in_=lvw_h[0:128, :])
                elif t == 31:
                    nc.gpsimd.dma_start(out=lv1[:], in_=lvw_h[128:256, :])
                pi, pj = t2piece[t]
                lw = mkw_sb[pi][:, pj * 128:(pj + 1) * 128]
                dve_tile = (t % 3 == 2)
                sim = ps_sim.tile([128, 1024], F32, tag="sim", name="sim")
                if not dve_tile:
                    for hh in (0, 1):
                        nc.tensor.matmul(sim[:, HOFF[hh]:HOFF[hh] + NH],
                                         ones128[:],
                                         nbsqz[:, hh * NH:(hh + 1) * NH],
                                         start=True, stop=False)
                for hh in (0, 1):
                    nc.tensor.matmul(sim[:, HOFF[hh]:HOFF[hh] + NH], lw,
                                     qs_sb[:, hh * NH:(hh + 1) * NH],
                                     start=dve_tile, stop=True)
                e = epool.tile([128, NCORE], BF16, tag="E")
                sim_ap = sim[:].rearrange("p (b x) -> p b x", b=2)[:, :, 0:NH]
                if dve_tile:
                    # b_sq subtraction on the DVE instead of init matmuls
                    dt_ = dpool.tile([128, NCORE], FP16, tag="D")
                    dt_ap = dt_[:].rearrange("p (b x) -> p b x", b=2)
                    bq_ap = bsqb[:].rearrange("p (b x) -> p b x", b=2)
                    nc.vector.tensor_add(dt_ap, sim_ap, bq_ap)
                    nc.scalar.activation(e[:], dt_[:], EXP,
                                         scale=ms_sb[:, t:t + 1])
                else:
                    e_ap = e[:].rearrange("p (b x) -> p b x", b=2)
                    nc.scalar.activation(e_ap, sim_ap, EXP,
                                         scale=ms_sb[:, t:t + 1])
                e_tiles[t] = e
            if (t < TT and t >= 2 and t % 2 == 0) or t == TT:
                s = spool.tile([128, NCORE], BF16, tag="S")
                nc.vector.tensor_add(s[:], e_tiles[t - 2][:],
                                     e_tiles[t - 1][:])
                if t == 2:
                    nc.vector.tensor_copy(zacc[:], s[:])
                else:
                    nc.vector.tensor_add(zacc[:], zacc[:], s[:])
            if t >= SKEW:
                tc_ = t - SKEW
                e = e_tiles.pop(tc_)
                st, sp = (tc_ == 0), (tc_ == TT - 1)
                for k in (0, 1):
                    lwk = mv_sb[:, tc_ * CV + k * 128:tc_ * CV + (k + 1) * 128]
                    for hh in (0, 1):
                        nc.tensor.matmul(r_acc[k, hh][:], lwk,
                                         e[:, hh * NH:(hh + 1) * NH],
                                         start=st, stop=sp)

        # ---- finalize ----------------------------------------------------
        lnz = fin.tile([1, NCORE], F32, tag="lnz")
        zp = ps_sim.tile([128, 1024], F32, tag="sim", name="zp")
        for hh in (0, 1):
            nc.tensor.matmul(zp[0:2, HOFF[hh]:HOFF[hh] + NH], ones_z[:],
                             zacc[:, hh * NH:(hh + 1) * NH],
                             start=True, stop=True)
        zp_ap = zp[0:1, :].rearrange("p (b x) -> p b x", b=2)[:, :, 0:NH]
        lnz_ap = lnz[:].rearrange("p (b x) -> p b x", b=2)
        nc.scalar.activation(lnz_ap, zp_ap, LOG)
        rz = fin.tile([1, NCORE], F32, tag="rz")
        nc.scalar.activation(rz[:], lnz[:], EXP, scale=-1.0)   # 1/Z
        w1 = fin.tile([1, NCORE], F32R, tag="w1")
        nc.vector.tensor_mul(w1[:], rz[:], p_sb[:])            # p / Z

        w1s = fin.tile([128, NCORE], F32, tag="w1s")
        for hh in (0, 1):
            s_ = slice(hh * NH, (hh + 1) * NH)
            wps = ps_sim.tile([128, 1024], F32, tag="sim", name=f"wps{hh}")
            nc.tensor.matmul(wps[:, 0:NH], ones_b[:], w1[:, s_],
                             start=True, stop=True)
            nc.scalar.activation(w1s[:, s_], wps[:, 0:NH],
                                 mybir.ActivationFunctionType.Copy)

        for k, lvt in ((0, lv0), (1, lv1)):
            o = fin.tile([128, NCORE], F32, tag="O", bufs=2)
            for hh in (0, 1):
                s_ = slice(hh * NH, (hh + 1) * NH)
                nc.vector.tensor_mul(o[:, s_], r_acc[k, hh][:], w1s[:, s_])
                nc.vector.tensor_add(o[:, s_], o[:, s_], lvt[:, s_])
                nc.sync.dma_start(out=out_h[k * 128:(k + 1) * 128, s_],
                                  in_=o[:, s_])

    nc.finalize()
    return nc


def _get_program():
    if "nc" not in _CACHE:
        _CACHE["nc"] = build_program()
    return _CACHE["nc"]


def _make_in_maps(query_key, query_selection, memory_key, memory_shrinkage,
                  msk_value, uncert_prob):
    qk = np.asarray(query_key, np.float32).reshape(B, CK, HW)
    qe = np.asarray(query_selection, np.float32).reshape(B, CK, HW)
    mk = np.asarray(memory_key, np.float32).reshape(B, CK, THW)
    ms = np.asarray(memory_shrinkage, np.float32).reshape(B, THW)
    mv = np.asarray(msk_value, np.float32).reshape(B, CV, THW)
    lv = np.asarray(msk_value, np.float32).reshape(B, CV, T, HW)[:, :, T - 1, :]
    p = np.asarray(uncert_prob, np.float32).reshape(B, HW)

    # per-batch device arrays (shared by the 4 n-shard cores of each batch)
    mkw_b, mv_b, ms_b = [], [], []
    for b in range(B):
        mkw_b.append(np.concatenate([mk[b] * mk[b], mk[b]],
                                    axis=0).astype(np.float16))
        mv_b.append(np.ascontiguousarray(mv[b].T).astype(np.float16))
        ms_b.append(np.ascontiguousarray(ms[b].reshape(TT, 128).T))

    qkqe = qk * qe                                   # B,CK,HW
    qs_full = np.concatenate([qe * (-0.125), qkqe * 0.25], axis=1)  # B,128,HW
    nbsq_full = (qkqe * qk).sum(axis=1) * (-0.125)   # B,HW
    lvw_full = lv * (1.0 - p[:, None, :])            # B,CV,HW

    in_maps = []
    for core in range(8):
        b, s = divmod(core, 4)
        sl = slice(s * NCORE, (s + 1) * NCORE)
        in_maps.append({
            "c_onesz": np.ones((128, 2), np.float32),
            "c_onesb": np.ones((1, 128), np.float32),
            "qs": np.ascontiguousarray(qs_full[b, :, sl]).astype(np.float16),
            "nbsq": np.ascontiguousarray(
                nbsq_full[b, sl]).astype(np.float16).reshape(1, NCORE),
            "mkw": mkw_b[b],
            "msT": ms_b[b],
            "mvT": mv_b[b],
            "lvw": np.ascontiguousarray(lvw_full[b, :, sl]),
            "p": np.ascontiguousarray(p[b, sl]).reshape(1, NCORE),
        })
    return in_maps


def kernel(**inputs):
    nc = _get_program()
    in_maps = _make_in_maps(**inputs)
    res = run_bass_kernel_spmd(nc, in_maps, list(range(8)))
    out = np.empty((B, 1, CV, HW), np.float32)
    for core in range(8):
        b, s = divmod(core, 4)
        out[b, 0, :, s * NCORE:(s + 1) * NCORE] = res.results[core]["out"]
    return out.reshape(B, 1, CV, H, W)


if __name__ == "__main__":
    rng = np.random.default_rng(0)
    dummy = {
        "query_key": rng.standard_normal((B, CK, H, W)).astype(np.float32),
        "query_selection": rng.random((B, CK, H, W)).astype(np.float32),
        "memory_key": rng.standard_normal((B, CK, T, H, W)).astype(np.float32),
        "memory_shrinkage": rng.random((B, 1, T, H, W)).astype(np.float32),
        "msk_value": rng.standard_normal((B, 1, CV, T, H, W)).astype(np.float32),
        "uncert_prob": rng.random((B, 1, H, W)).astype(np.float32),
    }
    out = kernel(**dummy)
    print("out", out.shape, out.dtype, float(np.abs(out).mean()))


# revision 23
# speedup vs baseline: 1.0448x; 1.0448x over previous
"""MatAnyone memory-readout kernel for 8 Trainium2 NeuronCores.

Math (per batch b):
  sim[t,n]  = (-a_sq + two_ab - b_sq)[t,n] * ms[t] / sqrt(CK)
  aff       = softmax_t(sim)
  R[c,n]    = sum_t mv[c,t] * aff[t,n]
  out[c,n]  = R[c,n] * p[n] + lv[c,n] * (1 - p[n])

Sharding: 8 cores = 2 batches x 4 query-pixel shards (n = HW/4 = 576 each).
Single interleaved pass over 144 t-tiles. Per tile the PE issues 8 N=288
matmuls (two per weight load, all K=128 so the PE never changes tiling mode):
2 psum inits seeding -b_sq/8 (all-ones lhsT against a zero-padded rhs whose
row 0 holds -b_sq/8), 2 sim accumulates with lhsT = [mk^2 ; mk] against
rhs [-qe/8 ; qe*qk/4], and 4 readout accumulates. The two sim halves live at
[0:288] and [512:800] of one two-bank psum tile so a single strided ACT
computes E = Exp(ms_t * psum) for the whole tile (per-partition scale, PSUM
source). The softmax denominator Z is accumulated on the DVE (pairwise bf16
adds at 2x + f32 accumulate). A manual load of ACT table set 6
(natural_log_exp_and_others) covers Exp AND Ln so the finalize's
1/Z = exp(-ln Z) causes no table switches. Finalize: Z column-sum matmul,
out = R * (p/Z) + host-folded lv*(1-p). Softmax max-subtraction is skipped:
sim <= 0 always (negative weighted L2 distance), so exp never overflows and
Z is well-scaled.
"""

import sys

for _p in ("/opt/trn_rl_repo", "/root/.axon_site/_ro/trn_rl_repo"):
    if _p not in sys.path:
        sys.path.insert(0, _p)

from contextlib import ExitStack

import numpy as np
import ml_dtypes

import concourse.bass as bass
from concourse import mybir
from concourse.bacc import Bacc
from concourse.tile import TileContext
from concourse.bass_utils import run_bass_kernel_spmd

F32 = mybir.dt.float32
F32R = mybir.dt.float32r
BF16 = mybir.dt.bfloat16
FP16 = mybir.dt.float16
EXP = mybir.ActivationFunctionType.Exp
LOG = mybir.ActivationFunctionType.Ln

B, CK, CV, T, H, W = 2, 64, 256, 8, 48, 48
HW = H * W            # 2304
THW = T * HW          # 18432
NCORE = HW // 4       # 576 query pixels per core
NH = NCORE // 2       # 288 per n-half (psum-bank sized)
TT = THW // 128       # 144 t-tiles
SKEW = 4              # software-pipeline skew (tiles) between exp and readout
ACT_SET_LN_EXP = 6    # natural_log_exp_and_others in act_info.json

# mkw DMA pieces (start_tile, n_tiles): small first piece so sims start early
MK_PIECES = [(0, 8), (8, 48), (56, 44), (100, 44)]

_CACHE = {}


def build_program():
    nc = Bacc(name="matanyone_knn")

    cz_h = nc.declare_dram_parameter("c_onesz", [128, 2], F32R, isOutput=False)
    cb_h = nc.declare_dram_parameter("c_onesb", [1, 128], F32R, isOutput=False)
    qs_h = nc.declare_dram_parameter("qs", [128, NCORE], FP16, isOutput=False)
    nb_h = nc.declare_dram_parameter("nbsqz", [128, NCORE], FP16, isOutput=False)
    mkw_h = nc.declare_dram_parameter("mkw", [128, THW], FP16, isOutput=False)
    ms_h = nc.declare_dram_parameter("msT", [128, TT], F32, isOutput=False)
    mv_h = nc.declare_dram_parameter("mvT", [THW, CV], FP16, isOutput=False)
    lvw_h = nc.declare_dram_parameter("lvw", [CV, NCORE], F32, isOutput=False)
    p_h = nc.declare_dram_parameter("p", [1, NCORE], F32, isOutput=False)
    out_h = nc.declare_dram_parameter("out", [CV, NCORE], F32, isOutput=True)

    with TileContext(nc) as tc, ExitStack() as ctx:
        persist = ctx.enter_context(tc.tile_pool(name="persist", bufs=1))
        mvpool = ctx.enter_context(tc.tile_pool(name="mv", bufs=1))
        mkpool = ctx.enter_context(tc.tile_pool(name="mkw", bufs=1))
        epool = ctx.enter_context(tc.tile_pool(name="E", bufs=SKEW + 4))
        spool = ctx.enter_context(tc.tile_pool(name="S", bufs=2))
        dpool = ctx.enter_context(tc.tile_pool(name="D", bufs=2))
        fin = ctx.enter_context(tc.tile_pool(name="fin", bufs=1))
        ps_sim = ctx.enter_context(tc.tile_pool(name="pssim", bufs=2, space="PSUM"))
        ps_acc = ctx.enter_context(tc.tile_pool(name="psacc", bufs=1, space="PSUM"))

        # preload the ACT table set that has BOTH Exp and Ln, so the
        # finalize's Ln/Exp trigger no mid-kernel table switches
        nc.scalar.add_instruction(mybir.InstLoadActFuncSet(
            name=nc.get_next_instruction_name(), ins=[], outs=[],
            act_func_set_id=ACT_SET_LN_EXP))

        # ---- streamed big weights on the gpsimd DMA queue ----------------
        mkw_sb = [mkpool.tile([128, n * 128], FP16, tag=f"mkw{i}",
                              name=f"mkw{i}")
                  for i, (s0, n) in enumerate(MK_PIECES)]

        def load_mkw_piece(i, eng=None):
            s0, n = MK_PIECES[i]
            (eng or nc.gpsimd).dma_start(
                out=mkw_sb[i][:], in_=mkw_h[:, s0 * 128:(s0 + n) * 128])

        mv_sb = mvpool.tile([128, TT * CV], FP16, tag="mvres")

        def load_mv_chunk(g):
            src = mv_h[g * 2048:(g + 1) * 2048, :].rearrange(
                "(j p) c -> p j c", p=128)
            dst = mv_sb[:, g * 16 * CV:(g + 1) * 16 * CV].rearrange(
                "p (j c) -> p j c", c=CV)
            nc.gpsimd.dma_start(out=dst, in_=src)

        # ---- constants generated on-device; small criticals first --------
        ones128 = persist.tile([128, 128], FP16, tag="ones128")  # init lhsT
        nc.vector.memset(ones128[:], 1.0)

        # critical small inputs ride the pre-loop-idle scalar DMA queue
        # (fast, big packets), ordered by first use; piece1 follows
        nbsqz = persist.tile([128, NCORE], FP16, tag="nbsqz")
        nc.scalar.dma_start(out=nbsqz[:], in_=nb_h[:])
        qs_sb = persist.tile([128, NCORE], FP16, tag="qs")
        nc.scalar.dma_start(out=qs_sb[:], in_=qs_h[:])
        load_mkw_piece(0, nc.scalar)
        ms_sb = persist.tile([128, TT], F32, tag="ms")
        nc.scalar.dma_start(out=ms_sb[:], in_=ms_h[:])
        load_mkw_piece(1, nc.scalar)

        load_mv_chunk(0)
        load_mv_chunk(1)

        ones_z = persist.tile([128, 2], F32R, tag="ones_z")      # Z matmul lhsT
        nc.sync.dma_start(out=ones_z[:], in_=cz_h[:])
        ones_b = persist.tile([1, 128], F32R, tag="ones_b")      # bcast lhsT
        nc.sync.dma_start(out=ones_b[:], in_=cb_h[:])
        p_sb = persist.tile([1, NCORE], F32, tag="p")
        nc.sync.dma_start(out=p_sb[:], in_=p_h[:])

        lv0 = fin.tile([128, NCORE], F32, tag="lv0")
        lv1 = fin.tile([128, NCORE], F32, tag="lv1")

        # -b_sq/8 broadcast to all partitions, for the DVE-subtract tiles
        bsqb = persist.tile([128, NCORE], F32, tag="bsqb")
        for hh in (0, 1):
            bp = ps_sim.tile([128, 1024], F32, tag="sim", name=f"bp{hh}")
            nc.tensor.matmul(bp[:, 0:NH], ones128[:],
                             nbsqz[:, hh * NH:(hh + 1) * NH],
                             start=True, stop=True)
            nc.vector.tensor_copy(bsqb[:, hh * NH:(hh + 1) * NH],
                                  bp[:, 0:NH])

        # ---- main interleaved pass --------------------------------------
        r_acc = {}
        for k in (0, 1):
            for hh in (0, 1):
                r_acc[k, hh] = ps_acc.tile([128, NH], F32, tag=f"r{k}{hh}",
                                           name=f"r{k}{hh}")

        zacc = persist.tile([128, NCORE], F32R, tag="zacc")

        # tile index -> (mkw piece, offset within piece)
        t2piece = {}
        for i, (s0, n) in enumerate(MK_PIECES):
            for j in range(n):
                t2piece[s0 + j] = (i, j)

        HOFF = (0, 512)   # column offsets of the two n-halves in sim psum

        e_tiles = {}
        for t in range(TT + SKEW):
            if t < TT:
                if t % 16 == 0 and 2 + t // 16 <= 8:
                    load_mv_chunk(2 + t // 16)
                if t == 20:
                    load_mkw_piece(2)
                elif t == 64:
                    load_mkw_piece(3)
                elif t == 30:
                    nc.gpsimd.dma_start(out=lv0[:], in_=lvw_h[0:128, :])
                elif t == 31:
                    nc.gpsimd.dma_start(out=lv1[:], in_=lvw_h[128:256, :])
                pi, pj = t2piece[t]
                lw = mkw_sb[pi][:, pj * 128:(pj + 1) * 128]
                dve_tile = (t % 4 == 2)
                sim = ps_sim.tile([128, 1024], F32, tag="sim", name="sim")
                if not dve_tile:
                    for hh in (0, 1):
                        nc.tensor.matmul(sim[:, HOFF[hh]:HOFF[hh] + NH],
                                         ones128[:],
                                         nbsqz[:, hh * NH:(hh + 1) * NH],
                                         start=True, stop=False)
                for hh in (0, 1):
                    nc.tensor.matmul(sim[:, HOFF[hh]:HOFF[hh] + NH], lw,
                                     qs_sb[:, hh * NH:(hh + 1) * NH],
                                     start=dve_tile, stop=True)
                e = epool.tile([128, NCORE], BF16, tag="E")
                sim_ap = sim[:].rearrange("p (b x) -> p b x", b=2)[:, :, 0:NH]
                if dve_tile:
                    # b_sq subtraction on the DVE instead of init matmuls
                    dt_ = dpool.tile([128, NCORE], FP16, tag="D")
                    dt_ap = dt_[:].rearrange("p (b x) -> p b x", b=2)
                    bq_ap = bsqb[:].rearrange("p (b x) -> p b x", b=2)
                    nc.vector.tensor_add(dt_ap, sim_ap, bq_ap)
                    nc.scalar.activation(e[:], dt_[:], EXP,
                                         scale=ms_sb[:, t:t + 1])
                else:
                    e_ap = e[:].rearrange("p (b x) -> p b x", b=2)
                    nc.scalar.activation(e_ap, sim_ap, EXP,
                                         scale=ms_sb[:, t:t + 1])
                e_tiles[t] = e
            if (t < TT and t >= 2 and t % 2 == 0) or t == TT:
                s = spool.tile([128, NCORE], BF16, tag="S")
                nc.vector.tensor_add(s[:], e_tiles[t - 2][:],
                                     e_tiles[t - 1][:])
                if t == 2:
                    nc.vector.tensor_copy(zacc[:], s[:])
                else:
                    nc.vector.tensor_add(zacc[:], zacc[:], s[:])
            if t >= SKEW:
                tc_ = t - SKEW
                e = e_tiles.pop(tc_)
                st, sp = (tc_ == 0), (tc_ == TT - 1)
                for k in (0, 1):
                    lwk = mv_sb[:, tc_ * CV + k * 128:tc_ * CV + (k + 1) * 128]
                    for hh in (0, 1):
                        nc.tensor.matmul(r_acc[k, hh][:], lwk,
                                         e[:, hh * NH:(hh + 1) * NH],
                                         start=st, stop=sp)

        # ---- finalize ----------------------------------------------------
        lnz = fin.tile([1, NCORE], F32, tag="lnz")
        zp = ps_sim.tile([128, 1024], F32, tag="sim", name="zp")
        for hh in (0, 1):
            nc.tensor.matmul(zp[0:2, HOFF[hh]:HOFF[hh] + NH], ones_z[:],
                             zacc[:, hh * NH:(hh + 1) * NH],
                             start=True, stop=True)
        zp_ap = zp[0:1, :].rearrange("p (b x) -> p b x", b=2)[:, :, 0:NH]
        lnz_ap = lnz[:].rearrange("p (b x) -> p b x", b=2)
        nc.scalar.activation(lnz_ap, zp_ap, LOG)
        rz = fin.tile([1, NCORE], F32, tag="rz")
        nc.scalar.activation(rz[:], lnz[:], EXP, scale=-1.0)   # 1/Z
        w1 = fin.tile([1, NCORE], F32R, tag="w1")
        nc.vector.tensor_mul(w1[:], rz[:], p_sb[:])            # p / Z

        w1s = fin.tile([128, NCORE], F32, tag="w1s")
        for hh in (0, 1):
            s_ = slice(hh * NH, (hh + 1) * NH)
            wps = ps_sim.tile([128, 1024], F32, tag="sim", name=f"wps{hh}")
            nc.tensor.matmul(wps[:, 0:NH], ones_b[:], w1[:, s_],
                             start=True, stop=True)
            nc.scalar.activation(w1s[:, s_], wps[:, 0:NH],
                                 mybir.ActivationFunctionType.Copy)

        for k, lvt in ((0, lv0), (1, lv1)):
            o = fin.tile([128, NCORE], F32, tag="O", bufs=2)
            for hh in (0, 1):
                s_ = slice(hh * NH, (hh + 1) * NH)
                nc.vector.tensor_mul(o[:, s_], r_acc[k, hh][:], w1s[:, s_])
                nc.vector.tensor_add(o[:, s_], o[:, s_], lvt[:, s_])
                nc.sync.dma_start(out=out_h[k * 128:(k + 1) * 128, s_],
                                  in_=o[:, s_])

    nc.finalize()
    return nc


def _get_program():
    if "nc" not in _CACHE:
        _CACHE["nc"] = build_program()
    return _CACHE["nc"]


def _make_in_maps(query_key, query_selection, memory_key, memory_shrinkage,
                  msk_value, uncert_prob):
    qk = np.asarray(query_key, np.float32).reshape(B, CK, HW)
    qe = np.asarray(query_selection, np.float32).reshape(B, CK, HW)
    mk = np.asarray(memory_key, np.float32).reshape(B, CK, THW)
    ms = np.asarray(memory_shrinkage, np.float32).reshape(B, THW)
    mv = np.asarray(msk_value, np.float32).reshape(B, CV, THW)
    lv = np.asarray(msk_value, np.float32).reshape(B, CV, T, HW)[:, :, T - 1, :]
    p = np.asarray(uncert_prob, np.float32).reshape(B, HW)

    # per-batch device arrays (shared by the 4 n-shard cores of each batch)
    mkw_b, mv_b, ms_b = [], [], []
    for b in range(B):
        mkw_b.append(np.concatenate([mk[b] * mk[b], mk[b]],
                                    axis=0).astype(np.float16))
        mv_b.append(np.ascontiguousarray(mv[b].T).astype(np.float16))
        ms_b.append(np.ascontiguousarray(ms[b].reshape(TT, 128).T))

    qkqe = qk * qe                                   # B,CK,HW
    qs_full = np.concatenate([qe * (-0.125), qkqe * 0.25], axis=1)  # B,128,HW
    nbsq_full = (qkqe * qk).sum(axis=1) * (-0.125)   # B,HW
    lvw_full = lv * (1.0 - p[:, None, :])            # B,CV,HW

    in_maps = []
    for core in range(8):
        b, s = divmod(core, 4)
        sl = slice(s * NCORE, (s + 1) * NCORE)
        nbsqz = np.zeros((128, NCORE), np.float16)
        nbsqz[0, :] = nbsq_full[b, sl].astype(np.float16)
        in_maps.append({
            "c_onesz": np.ones((128, 2), np.float32),
            "c_onesb": np.ones((1, 128), np.float32),
            "qs": np.ascontiguousarray(qs_full[b, :, sl]).astype(np.float16),
            "nbsqz": nbsqz,
            "mkw": mkw_b[b],
            "msT": ms_b[b],
            "mvT": mv_b[b],
            "lvw": np.ascontiguousarray(lvw_full[b, :, sl]),
            "p": np.ascontiguousarray(p[b, sl]).reshape(1, NCORE),
        })
    return in_maps


def kernel(**inputs):
    nc = _get_program()
    in_maps = _make_in_maps(**inputs)
    res = run_bass_kernel_spmd(nc, in_maps, list(range(8)))
    out = np.empty((B, 1, CV, HW), np.float32)
    for core in range(8):
        b, s = divmod(core, 4)
        out[b, 0, :, s * NCORE:(s + 1) * NCORE] = res.results[core]["out"]
    return out.reshape(B, 1, CV, H, W)


if __name__ == "__main__":
    rng = np.random.default_rng(0)
    dummy = {
        "query_key": rng.standard_normal((B, CK, H, W)).astype(np.float32),
        "query_selection": rng.random((B, CK, H, W)).astype(np.float32),
        "memory_key": rng.standard_normal((B, CK, T, H, W)).astype(np.float32),
        "memory_shrinkage": rng.random((B, 1, T, H, W)).astype(np.float32),
        "msk_value": rng.standard_normal((B, 1, CV, T, H, W)).astype(np.float32),
        "uncert_prob": rng.random((B, 1, H, W)).astype(np.float32),
    }
    out = kernel(**dummy)
    print("out", out.shape, out.dtype, float(np.abs(out).mean()))
